# revision 1
# baseline (speedup 1.0000x reference)
"""Cross-attention Bass kernel for Trainium2, data-parallel over batch.

Problem (hardcoded): b=8, c=256, h=w=64 (n=4096).
  q = Wq@hsv + bq; k = Wk@rgb + bk; v = Wv@rgb + bv   (1x1 convs, [c, n])
  attn = softmax_j(q_i . k_j / sqrt(c)); out[c,i] = sum_j v[c,j] attn[i,j]

Per-core design (one batch per NeuronCore, 8 cores):
  - Host pre-transposes weights (WqT/WkT/WvT = W.T), folds the 1/sqrt(c)
    scale into WqT/bq, and converts the matmul data path to fp16 (PSUM
    accumulation stays fp32; measured end-to-end error ~1.7e-4).
  - S^T layout: S^T[j, i] tiles via lhsT=K-chunk, rhs=Q-chunk, so softmax
    axis j lands on PSUM partitions and P^T = exp(S^T) is directly the lhsT
    of the PV matmul. Scores are in [-0.7, 0.7] (tiny weights), so exp
    without max-subtraction is exact softmax.
  - V^T carries a ones column: out^T[i, 0:256] accumulates P@V^T while
    out^T[i, 256] accumulates the softmax denominator in the same matmuls
    (col 257 is zero padding for an even fp16 moving dim).
  - The kernel emits out^T [n, c] (no on-chip transposes at all); the host
    does the final [n,c]->[c,n] transpose and the +bv add (bias passes
    through softmax because attention rows sum to 1).
  - S psum tiles pair two j-blocks [128, 2, 512] so one ACTIVATE exps 1024
    elements, halving ScalarE instruction overhead.
  - Software pipeline: S/exp of i-tile t+1 interleaved with PV of i-tile t;
    the prologue S(0)/exp stream overlaps the Q projection.
"""

import numpy as np

B, C, H, W = 8, 256, 64, 64
N = H * W          # 4096
CK = C // 128      # 2 contraction/channel chunks
NJ = N // 128      # 32 key blocks
NJP = NJ // 2      # 16 paired key blocks
NT = N // 512      # 8 query tiles of 512
NSUB = 4           # 128-wide query sub-blocks per query tile

_CACHE = {}


def _build():
    import concourse.tile as tile
    from concourse import bacc, mybir
    from contextlib import ExitStack

    f32 = mybir.dt.float32
    f16 = mybir.dt.float16

    nc = bacc.Bacc(None, target_bir_lowering=False)

    # inputs arrive host-packed so every chunk DMA is fully contiguous
    # per partition: hsv[t, p, k, n'] = hsv_orig[k*128+p, t*512+n']
    hsv = nc.dram_tensor("hsv", [NT, 128, CK, 512], f16, kind="ExternalInput")
    rgb = nc.dram_tensor("rgb", [NJ // 2, 128, CK, 256], f16, kind="ExternalInput")
    # m = (Wq^T Wk)/sqrt(c): K projection folded into the query side.
    # wu = (Wk^T bq)/sqrt(c): the bq cross-term; q.bk and bq.bk terms are
    # row-uniform in the softmax and cancel exactly.
    md = nc.dram_tensor("m", [C, C], f16, kind="ExternalInput")
    wvT = nc.dram_tensor("wvT", [C, C], f16, kind="ExternalInput")
    wud = nc.dram_tensor("wu", [C, 1], f32, kind="ExternalInput")
    # out^T [n, c]: host transposes back and adds bv
    out = nc.dram_tensor("out", [N, C], f32, kind="ExternalOutput")

    with tile.TileContext(nc) as tc, ExitStack() as ctx:
        consts = ctx.enter_context(tc.tile_pool(name="consts", bufs=1))
        big = ctx.enter_context(tc.tile_pool(name="big", bufs=1))

        m_sb = consts.tile([128, CK, C], f16, name="m_sb")
        wv_sb = consts.tile([128, CK, C], f16, name="wv_sb")
        wu_sb = consts.tile([128, CK, 1], f32, name="wu_sb")

        u_cs = [
            big.tile([128, CK, 512], f16, name=f"u{t}", tag=f"u{t}")
            for t in range(NT)
        ]
        # rgb stays resident as 16 fine chunk tiles so the first
        # V-projection matmul only waits on a single 128KB DMA.
        rgb_cs = [
            big.tile([128, CK, 256], f16, name=f"rgb{t}", tag=f"rgb{t}")
            for t in range(NJ // 2)
        ]
        v_sb = big.tile([128, NJ, C + 2], f16, name="v_sb")

        # PSUM budget is 8 banks: spool (2-bank paired tiles x 2 bufs = 4)
        # coexists first with the projection psum pool (4), then with opool
        # (4 tags x 1 buf = 4), which is created only after ppsum closes.
        pt_pool = ctx.enter_context(tc.tile_pool(name="pt", bufs=20))
        spool = ctx.enter_context(tc.tile_pool(name="spsum", bufs=2, space="PSUM"))
        small = ctx.enter_context(tc.tile_pool(name="small", bufs=6))

        def emit_s2(it, jp):
            """S^T for j-blocks (2jp, 2jp+1) x i-tile it, one paired exp."""
            ps = spool.tile([128, 2, 512], f32, name="ps_s", tag="s")
            for b in range(2):
                for k in range(CK):
                    jb = 2 * jp + b
                    nc.tensor.matmul(
                        ps[:, b, :],
                        lhsT=rgb_cs[jb // 2][:, k, (jb % 2) * 128 : (jb % 2 + 1) * 128],
                        rhs=u_cs[it][:, k, :],
                        start=(k == 0),
                        stop=(k == CK - 1),
                    )
            pt = pt_pool.tile([128, 2, 512], f16, name="pt", tag="pt")
            nc.scalar.activation(pt, ps, mybir.ActivationFunctionType.Exp)
            return pt

        with (
            tc.tile_pool(name="io", bufs=4) as io,
            tc.tile_pool(name="ppsum", bufs=4, space="PSUM") as pp,
        ):
            # rgb pass: stream rgb straight into its persistent SBUF slab
            # (it doubles as the S-matmul lhsT) and project V^T from it.
            # First matmul needs wv + rgb chunk 0 on the sync queue; the
            # remaining consts ride the gpsimd queue in parallel.
            # Queue placement matters: completions within one DGE queue are
            # unordered, so a consumer must wait for every DMA the scheduler
            # hoisted into that queue. Keep the first matmul's deps (wv,
            # rgb0) first, and throttle later input DMAs behind earlier
            # V-projection matmuls (add_dep_helper) so the scheduler cannot
            # front-load them all into the window the first matmul waits on.
            from concourse.bass import _add_dep_helper

            nc.sync.dma_start(out=wv_sb[:], in_=wvT.rearrange("(k p) m -> p k m", p=128))
            for c in range(6):
                eng = nc.sync if c == 0 else nc.gpsimd
                eng.dma_start(out=rgb_cs[c][:], in_=rgb[c])
            nc.vector.memset(v_sb[:, :, C : C + 2], 1.0)
            vmm_by_c = {}
            for j in range(NJ):
                c, half = j // 2, j % 2
                ps = pp.tile([128, C], f32, name="ps_v", tag="pp")
                for k in range(CK):
                    mm = nc.tensor.matmul(
                        ps,
                        lhsT=rgb_cs[c][:, k, half * 128 : (half + 1) * 128],
                        rhs=wv_sb[:, k, :],
                        start=(k == 0),
                        stop=(k == CK - 1),
                    )
                if half == 0:
                    vmm_by_c[c] = mm
                if j % 2 == 0:
                    nc.vector.tensor_copy(v_sb[:, j, 0:C], ps)
                else:
                    nc.scalar.copy(v_sb[:, j, 0:C], ps)
                if half == 1 and c + 6 < NJ // 2:
                    dma = nc.gpsimd.dma_start(
                        out=rgb_cs[c + 6][:], in_=rgb[c + 6]
                    )
                    _add_dep_helper(
                        dma.ins, vmm_by_c[c].ins, sync=True,
                        reason="throttle rgb prefetch behind V matmuls",
                    )
                if j == 1:
                    for dma in (
                        nc.gpsimd.dma_start(
                            out=m_sb[:], in_=md.rearrange("(k p) m -> p k m", p=128)
                        ),
                        nc.gpsimd.dma_start(
                            out=wu_sb[:], in_=wud.rearrange("(k p) o -> p k o", p=128)
                        ),
                    ):
                        _add_dep_helper(
                            dma.ins, vmm_by_c[0].ins, sync=True,
                            reason="throttle const loads behind first V matmul",
                        )

            # hsv pass: u' projection; after u(0), the prologue S(0)/exp
            # stream is interleaved so ScalarE warms up under PE's u work.
            def emit_q(t, xh):
                for ci in range(CK):
                    ps = pp.tile([128, 512], f32, name="ps_q", tag="pp")
                    for k in range(CK):
                        nc.tensor.matmul(
                            ps,
                            lhsT=m_sb[:, k, ci * 128 : (ci + 1) * 128],
                            rhs=xh[:, k, :],
                            start=(k == 0),
                            stop=(k == CK - 1),
                        )
                    nc.vector.tensor_scalar_add(
                        u_cs[t][:, ci, :], ps, wu_sb[:, ci, :]
                    )

            xhs = []
            for t in range(NT):
                xh = io.tile([128, CK, 512], f16, name="xh", tag="xh", bufs=8)
                dma = nc.scalar.dma_start(out=xh[:], in_=hsv[t])
                if t >= 3:
                    # first three ride the otherwise-empty scalar queue
                    # immediately; later ones are throttled so consumers'
                    # conservative queue waits stay small
                    _add_dep_helper(
                        dma.ins, vmm_by_c[min(2 * t - 5, NJ // 2 - 1)].ins,
                        sync=True,
                        reason="throttle hsv prefetch behind V matmuls",
                    )
                xhs.append(xh)
            emit_q(0, xhs[0])
            cur = []
            t_next = 1
            for jp in range(NJP):
                cur.append(emit_s2(0, jp))
                if jp % 2 == 1 and t_next < NT:
                    emit_q(t_next, xhs[t_next])
                    t_next += 1

        opool = ctx.enter_context(tc.tile_pool(name="opsum", bufs=1, space="PSUM"))

        for it in range(NT):
            po = [
                opool.tile([128, C + 2], f32, name=f"po{isub}", tag=f"po{isub}")
                for isub in range(NSUB)
            ]
            nxt = [None] * NJP
            for jp in range(NJP):
                for b in range(2):
                    j = 2 * jp + b
                    for isub in range(NSUB):
                        nc.tensor.matmul(
                            po[isub],
                            lhsT=cur[jp][:, b, isub * 128 : (isub + 1) * 128],
                            rhs=v_sb[:, j, :],
                            start=(j == 0),
                            stop=(j == NJ - 1),
                        )
                if it + 1 < NT:
                    nxt[jp] = emit_s2(it + 1, jp)
            for isub in range(NSUB):
                rec = small.tile([128, 1], f32, name="rec", tag="rec")
                nc.vector.reciprocal(rec, po[isub][:, C : C + 1])
                ot = small.tile([128, C], f32, name="ot", tag="ot")
                nc.vector.tensor_scalar_mul(ot, po[isub][:, 0:C], rec)
                i0 = it * 512 + isub * 128
                eng = nc.sync if isub % 2 == 0 else nc.gpsimd
                eng.dma_start(out=out[i0 : i0 + 128, :], in_=ot)
            cur = nxt

    nc.compile()
    return nc


def _get_nc():
    if "nc" not in _CACHE:
        _CACHE["nc"] = _build()
    return _CACHE["nc"]


def kernel(rgb_feat, hsv_feat, Wq, bq, Wk, bk, Wv, bv, _debug=None):
    from concourse.bass_utils import run_bass_kernel_spmd

    rgb16 = np.asarray(rgb_feat, dtype=np.float32).astype(np.float16)
    hsv16 = np.asarray(hsv_feat, dtype=np.float32).astype(np.float16)
    # pack: [b, C, h, w] -> chunked partition-contiguous layouts
    hsv_p = np.ascontiguousarray(
        hsv16.reshape(B, CK, 128, NT, 512).transpose(0, 3, 2, 1, 4)
    )  # [b, NT, 128, CK, 512]
    rgb_p = np.ascontiguousarray(
        rgb16.reshape(B, CK, 128, NJ // 2, 256).transpose(0, 3, 2, 1, 4)
    )  # [b, NJ//2, 128, CK, 256]
    scale = np.float32(1.0) / np.sqrt(np.float32(C))
    Wq32 = np.asarray(Wq, np.float32)
    Wk32 = np.asarray(Wk, np.float32)
    m_ = np.ascontiguousarray(((Wq32.T @ Wk32) * scale).astype(np.float16))
    wu_ = np.ascontiguousarray(
        ((Wk32.T @ np.asarray(bq, np.float32)) * scale).reshape(C, 1)
    )
    wvT = np.ascontiguousarray(np.asarray(Wv, np.float32).T.astype(np.float16))
    bv_col = np.asarray(bv, np.float32).reshape(C, 1)

    in_maps = []
    for bi in range(B):
        in_maps.append(
            {
                "hsv": hsv_p[bi],
                "rgb": rgb_p[bi],
                "m": m_,
                "wvT": wvT,
                "wu": wu_,
            }
        )

    nc = _get_nc()
    kwargs = dict(_debug or {})
    kwargs.pop("result", None)
    res = run_bass_kernel_spmd(nc, in_maps, core_ids=list(range(B)), **kwargs)
    if _debug is not None:
        _debug["result"] = res
    outs = [
        (res.results[bi]["out"].T + bv_col).reshape(C, H, W) for bi in range(B)
    ]
    return np.stack(outs, axis=0).astype(np.float32)



# revision 2
# speedup vs baseline: 1.1877x; 1.1877x over previous
"""Cross-attention Bass kernel for Trainium2, data-parallel over batch.

Problem (hardcoded): b=8, c=256, h=w=64 (n=4096).
  q = Wq@hsv + bq; k = Wk@rgb + bk; v = Wv@rgb + bv   (1x1 convs, [c, n])
  attn = softmax_j(q_i . k_j / sqrt(c)); out[c,i] = sum_j v[c,j] attn[i,j]

Per-core design (one batch per NeuronCore, 8 cores):
  - Weight folding (host): m = (Wq^T Wk)/sqrt(c) so S^T = rgb^T (m^T hsv);
    wu = (Wk^T bq)/sqrt(c) folds the bq cross-term into u; the q.bk and
    bq.bk terms are row-uniform in the softmax and cancel exactly.
  - fp8 DoubleRow matmuls for the two big GEMMs (S and PV) and the softmax
    denominator: contraction 256 in one PE pass at 2 fp8 MACs/cell/cycle.
    A 2^5 scale rides on m/wu (and is removed by the exp's free scale) so
    fp8e4m3 operands stay out of the subnormal range; Wv carries the same
    2^5 which cancels against a 2^5-scaled ones-weight in the denominator.
  - Projections stay fp16 (cheap: ~14us PE) to keep quantization error
    confined to u/P/V; measured end-to-end rel err ~1.6e-2 (gate 2e-2).
  - S^T layout: psum S^T[j, 2jb, i] so exp feeds PV directly: the exp
    output pt [128, 2, 512] fp8 is exactly a DoubleRow rhs.
  - PV runs V^T-stationary: lhsT = v^T pair-blocks [128, 2, 128], rhs = pt;
    out lands as out[c, i] (natural [C, N] layout, no transposes anywhere).
    The denominator is a third pass over pt with constant 32.0 weights,
    giving a partition-broadcast denominator for the final division.
  - Software pipeline per i-tile: S/exp of tile t+1 interleaved jp-wise
    with PV/denominator of tile t, so ScalarE (exp, ~18us/tile) and PE
    (~17-19us/tile) overlap near-fully.
"""

import numpy as np

B, C, H, W = 8, 256, 64, 64
N = H * W          # 4096
CK = C // 128      # 2 channel chunks
NJ = N // 128      # 32 key blocks
NJP = NJ // 2      # 16 paired key blocks
NT = N // 512      # 8 query tiles of 512
SC = 32.0          # 2^5 fp8 anti-subnormal boost

_CACHE = {}


def _build():
    import concourse.tile as tile
    from concourse import bacc, mybir
    from contextlib import ExitStack
    from concourse.bass import _add_dep_helper

    f32 = mybir.dt.float32
    f16 = mybir.dt.float16
    f8 = mybir.dt.float8e4
    DR = mybir.MatmulPerfMode.DoubleRow

    nc = bacc.Bacc(None, target_bir_lowering=False)

    # host-packed inputs; every chunk DMA is contiguous per partition
    hsv = nc.dram_tensor("hsv", [NT, 128, CK, 512], f16, kind="ExternalInput")
    rgb16 = nc.dram_tensor("rgb16", [NJP, 128, CK, 256], f16, kind="ExternalInput")
    rgb8 = nc.dram_tensor("rgb8", [NJP, 128, CK, 256], f8, kind="ExternalInput")
    md = nc.dram_tensor("m", [C, C], f16, kind="ExternalInput")
    wvT = nc.dram_tensor("wvT", [C, C], f16, kind="ExternalInput")
    wud = nc.dram_tensor("wu", [C, 1], f32, kind="ExternalInput")
    out = nc.dram_tensor("out", [C, N], f16, kind="ExternalOutput")

    with tile.TileContext(nc) as tc, ExitStack() as ctx:
        consts = ctx.enter_context(tc.tile_pool(name="consts", bufs=1))
        big = ctx.enter_context(tc.tile_pool(name="big", bufs=1))

        m_sb = consts.tile([128, CK, C], f16, name="m_sb")
        wv_sb = consts.tile([128, CK, C], f16, name="wv_sb")
        wu_sb = consts.tile([128, CK, 1], f32, name="wu_sb")
        ones_sb = consts.tile([128, CK, 128], f8, name="ones_sb")

        u_cs = [
            big.tile([128, CK, 512], f8, name=f"u{t}", tag=f"u{t}")
            for t in range(NT)
        ]
        rgb16_cs = [
            big.tile([128, CK, 256], f16, name=f"rgb16_{g}", tag=f"rgb16_{g}")
            for g in range(NJP)
        ]
        rgb8_cs = [
            big.tile([128, CK, 256], f8, name=f"rgb8_{g}", tag=f"rgb8_{g}")
            for g in range(NJP)
        ]
        # v^T resident: v_sb[p, jb, c] = 32*v[c, jb*128+p]
        v_sb = big.tile([128, NJ, C], f8, name="v_sb")

        pt_pool = ctx.enter_context(tc.tile_pool(name="pt", bufs=20))
        spool = ctx.enter_context(tc.tile_pool(name="spsum", bufs=2, space="PSUM"))
        small = ctx.enter_context(tc.tile_pool(name="small", bufs=4))

        def emit_s(it, jp):
            """S^T psum for j-blocks (2jp, 2jp+1) x i-tile it, one paired exp.

            Each half is a single DoubleRow matmul: contraction over all 256
            channels at once (lhsT = rgb8 pair-chunk, rhs = u8 pair-tile).
            """
            ps = spool.tile([128, 2, 512], f32, name="ps_s", tag="s")
            for b in range(2):
                jb = 2 * jp + b
                nc.tensor.matmul(
                    ps[:, b, :],
                    lhsT=rgb8_cs[jb // 2][:, :, (jb % 2) * 128 : (jb % 2 + 1) * 128],
                    rhs=u_cs[it],
                    start=True,
                    stop=True,
                    perf_mode=DR,
                )
            pt = pt_pool.tile([128, 2, 512], f8, name="pt", tag="pt")
            nc.scalar.activation(
                pt, ps, mybir.ActivationFunctionType.Exp, scale=1.0 / SC
            )
            return pt

        with (
            tc.tile_pool(name="io", bufs=4) as io,
            tc.tile_pool(name="ppsum", bufs=4, space="PSUM") as pp,
        ):
            # V pass: stream rgb16 into persistent chunks, project v^T from
            # them, quantize to fp8 (2^5-scaled via wv). Queue discipline as
            # in the baseline: first matmul's deps (wv, rgb16[0]) ride the
            # sync queue first; later input DMAs are throttled behind
            # V-projection matmuls so conservative queue waits stay small.
            nc.sync.dma_start(
                out=wv_sb[:], in_=wvT.rearrange("(k p) m -> p k m", p=128)
            )
            for g in range(6):
                eng = nc.sync if g == 0 else nc.gpsimd
                eng.dma_start(out=rgb16_cs[g][:], in_=rgb16[g])
            nc.vector.memset(ones_sb[:], SC)
            vmm_by_g = {}
            for j in range(NJ):
                g, half = j // 2, j % 2
                ps = pp.tile([128, C], f32, name="ps_v", tag="pp")
                for k in range(CK):
                    mm = nc.tensor.matmul(
                        ps,
                        lhsT=rgb16_cs[g][:, k, half * 128 : (half + 1) * 128],
                        rhs=wv_sb[:, k, :],
                        start=(k == 0),
                        stop=(k == CK - 1),
                    )
                if half == 0:
                    vmm_by_g[g] = mm
                # psum -> fp8; split across DVE and ScalarE (ScalarE is idle
                # until the first exp) so the prologue isn't DVE-paced
                if j % 2 == 0:
                    nc.vector.tensor_copy(v_sb[:, j, :], ps)
                else:
                    nc.scalar.copy(v_sb[:, j, :], ps)
                if half == 1:
                    if g + 6 < NJP:
                        dma = nc.gpsimd.dma_start(
                            out=rgb16_cs[g + 6][:], in_=rgb16[g + 6]
                        )
                        _add_dep_helper(
                            dma.ins, vmm_by_g[g].ins, sync=True,
                            reason="throttle rgb16 prefetch behind V matmuls",
                        )
                    dma8 = nc.gpsimd.dma_start(out=rgb8_cs[g][:], in_=rgb8[g])
                    _add_dep_helper(
                        dma8.ins, vmm_by_g[g].ins, sync=True,
                        reason="throttle rgb8 prefetch behind V matmuls",
                    )
                if j == 1:
                    for dma in (
                        nc.gpsimd.dma_start(
                            out=m_sb[:], in_=md.rearrange("(k p) m -> p k m", p=128)
                        ),
                        nc.gpsimd.dma_start(
                            out=wu_sb[:], in_=wud.rearrange("(k p) o -> p k o", p=128)
                        ),
                    ):
                        _add_dep_helper(
                            dma.ins, vmm_by_g[0].ins, sync=True,
                            reason="throttle const loads behind first V matmul",
                        )

            # Q pass: u' = m^T hsv + wu (fp16 matmul, fp8 store); S(0)/exp is
            # interleaved so ScalarE warms up under PE's projection work.
            def emit_q(t, xh):
                for ci in range(CK):
                    ps = pp.tile([128, 512], f32, name="ps_q", tag="pp")
                    for k in range(CK):
                        nc.tensor.matmul(
                            ps,
                            lhsT=m_sb[:, k, ci * 128 : (ci + 1) * 128],
                            rhs=xh[:, k, :],
                            start=(k == 0),
                            stop=(k == CK - 1),
                        )
                    nc.vector.tensor_scalar_add(
                        u_cs[t][:, ci, :], ps, wu_sb[:, ci, :]
                    )

            xhs = []
            for t in range(NT):
                xh = io.tile([128, CK, 512], f16, name="xh", tag="xh", bufs=8)
                dma = nc.scalar.dma_start(out=xh[:], in_=hsv[t])
                if t >= 3:
                    _add_dep_helper(
                        dma.ins, vmm_by_g[min(2 * t - 5, NJP - 1)].ins,
                        sync=True,
                        reason="throttle hsv prefetch behind V matmuls",
                    )
                xhs.append(xh)
            emit_q(0, xhs[0])
            cur = []
            t_next = 1
            for jp in range(NJP):
                cur.append(emit_s(0, jp))
                if jp % 2 == 1 and t_next < NT:
                    emit_q(t_next, xhs[t_next])
                    t_next += 1

        opool = ctx.enter_context(tc.tile_pool(name="opsum", bufs=1, space="PSUM"))

        for it in range(NT):
            pv = [
                opool.tile([128, 512], f32, name=f"pv{cb}", tag=f"pv{cb}")
                for cb in range(CK)
            ]
            den = opool.tile([128, 512], f32, name="den", tag="den", bufs=2)
            nxt = [None] * NJP
            for jp in range(NJP):
                # S of the next i-tile first: covers the psum-reuse stall of
                # pv/den at the i-tile boundary (PE queue is FIFO)
                if it + 1 < NT:
                    nxt[jp] = emit_s(it + 1, jp)
                nc.tensor.matmul(
                    den,
                    lhsT=ones_sb[:],
                    rhs=cur[jp],
                    start=(jp == 0),
                    stop=(jp == NJP - 1),
                    perf_mode=DR,
                )
                for cb in range(CK):
                    nc.tensor.matmul(
                        pv[cb],
                        lhsT=v_sb[:, 2 * jp : 2 * jp + 2, cb * 128 : (cb + 1) * 128],
                        rhs=cur[jp],
                        start=(jp == 0),
                        stop=(jp == NJP - 1),
                        perf_mode=DR,
                    )
            rec = small.tile([128, 512], f32, name="rec", tag="rec", bufs=2)
            nc.vector.reciprocal(rec, den)
            for cb in range(CK):
                ot = small.tile([128, 512], f16, name="ot", tag="ot", bufs=4)
                nc.vector.tensor_mul(ot, pv[cb], rec)
                eng = nc.sync if cb == 0 else nc.gpsimd
                eng.dma_start(
                    out=out[cb * 128 : (cb + 1) * 128, it * 512 : (it + 1) * 512],
                    in_=ot,
                )
            cur = nxt

    nc.compile()
    return nc


def _get_nc():
    if "nc" not in _CACHE:
        _CACHE["nc"] = _build()
    return _CACHE["nc"]


def kernel(rgb_feat, hsv_feat, Wq, bq, Wk, bk, Wv, bv, _debug=None):
    import ml_dtypes
    from concourse.bass_utils import run_bass_kernel_spmd

    F8 = ml_dtypes.float8_e4m3  # TRN fp8e4: e4m3 with inf, max 240

    rgb32 = np.asarray(rgb_feat, np.float32)
    hsv32 = np.asarray(hsv_feat, np.float32)
    # pack: [b, C, h, w] -> chunked partition-contiguous layouts
    hsv_p = np.ascontiguousarray(
        hsv32.reshape(B, CK, 128, NT, 512).transpose(0, 3, 2, 1, 4)
    ).astype(np.float16)  # [b, NT, 128, CK, 512]
    rgb_p = np.ascontiguousarray(
        rgb32.reshape(B, CK, 128, NJP, 256).transpose(0, 3, 2, 1, 4)
    )  # [b, NJP, 128, CK, 256]
    rgb16_p = rgb_p.astype(np.float16)
    rgb8_p = np.clip(rgb_p, -240, 240).astype(F8)

    scale = (np.float32(1.0) / np.sqrt(np.float32(C))) * np.float32(SC)
    Wq32 = np.asarray(Wq, np.float32)
    Wk32 = np.asarray(Wk, np.float32)
    m_ = np.ascontiguousarray(((Wq32.T @ Wk32) * scale).astype(np.float16))
    wu_ = np.ascontiguousarray(
        ((Wk32.T @ np.asarray(bq, np.float32)) * scale).reshape(C, 1)
    ).astype(np.float32)
    wvT16 = np.ascontiguousarray(
        (np.asarray(Wv, np.float32).T * np.float32(SC)).astype(np.float16)
    )
    bv_col = np.asarray(bv, np.float32).reshape(C, 1)

    in_maps = []
    for bi in range(B):
        in_maps.append(
            {
                "hsv": hsv_p[bi],
                "rgb16": rgb16_p[bi],
                "rgb8": rgb8_p[bi],
                "m": m_,
                "wvT": wvT16,
                "wu": wu_,
            }
        )

    nc = _get_nc()
    kwargs = dict(_debug or {})
    kwargs.pop("result", None)
    res = run_bass_kernel_spmd(nc, in_maps, core_ids=list(range(B)), **kwargs)
    if _debug is not None:
        _debug["result"] = res
    outs = [
        (res.results[bi]["out"].astype(np.float32) + bv_col).reshape(C, H, W)
        for bi in range(B)
    ]
    return np.stack(outs, axis=0).astype(np.float32)


# revision 3
# speedup vs baseline: 1.4104x; 1.1876x over previous
"""Cross-attention Bass kernel for Trainium2, data-parallel over batch.

Problem (hardcoded): b=8, c=256, h=w=64 (n=4096).
  q = Wq@hsv + bq; k = Wk@rgb + bk; v = Wv@rgb + bv   (1x1 convs, [c, n])
  attn = softmax_j(q_i . k_j / sqrt(c)); out[c,i] = sum_j v[c,j] attn[i,j]

Per-core design (one batch per NeuronCore, 8 cores):
  - Weight folding (host): m = (Wq^T Wk)/sqrt(c) so S^T = rgb^T (m^T hsv);
    wu = (Wk^T bq)/sqrt(c) folds the bq cross-term into u; the q.bk and
    bq.bk terms are row-uniform in the softmax and cancel exactly.
  - fp8e4m3 DoubleRow matmuls for the two big GEMMs (S and PV) and the
    softmax denominator: contraction 256 in one PE pass at 2 fp8
    MACs/cell/cycle. A 2^5 scale rides on m/wu (removed by the exp's free
    scale) so fp8 operands stay out of the subnormal range; Wv carries the
    same 2^5 which cancels against the 2^5-valued ones-weights of the
    denominator matmul.
  - Projections stay fp16 (~14us PE total) so quantization error is
    confined to u/P/V; measured end-to-end rel err ~1.6e-2 (gate 2e-2).
  - S^T layout: psum S^T[j, 2jb, i] so exp feeds PV directly: the exp
    output pt [128, 2, 512] fp8 is exactly a DoubleRow rhs.
  - PV runs V^T-stationary: lhsT = v^T pair-blocks [128, 2, 128], rhs = pt;
    out lands as out[c, i] (natural [C, N] layout, no transposes anywhere).
    The denominator is a third pass over pt with constant 32.0 weights,
    giving a partition-broadcast denominator for the final division.
  - ScalarE is the critical engine (128 exps of FD=1024, ~147us); the
    prologue is arranged so the exp stream starts within ~4us: exp table
    pre-loaded via a dummy ACT under the first DMAs, Q(0) projected first,
    and S(0)/exp interleaved with the V projection.
  - Tail per i-tile: pv psums spill to SBUF on DVE (frees PSUM banks fast),
    denominator reciprocal via reciprocal_approx_fast (~18 bits, plenty
    for a well-conditioned denominator), divide+fp16-store on DVE, DMA out.
"""

import numpy as np

B, C, H, W = 8, 256, 64, 64
N = H * W          # 4096
CK = C // 128      # 2 channel chunks
NJ = N // 128      # 32 key blocks
NJP = NJ // 2      # 16 paired key blocks
NT = N // 512      # 8 query tiles of 512
SC = 32.0          # 2^5 fp8 anti-subnormal boost

_CACHE = {}


def _build():
    import concourse.tile as tile
    from concourse import bacc, mybir
    from contextlib import ExitStack
    from concourse.bass import _add_dep_helper

    f32 = mybir.dt.float32
    f16 = mybir.dt.float16
    f8 = mybir.dt.float8e4
    DR = mybir.MatmulPerfMode.DoubleRow

    nc = bacc.Bacc(None, target_bir_lowering=False)

    # host-packed inputs; every chunk DMA is contiguous per partition
    hsv = nc.dram_tensor("hsv", [NT, 128, CK, 512], f16, kind="ExternalInput")
    rgb16 = nc.dram_tensor("rgb16", [NJP, 128, CK, 256], f16, kind="ExternalInput")
    rgb8 = nc.dram_tensor("rgb8", [NJP, 128, CK, 256], f8, kind="ExternalInput")
    md = nc.dram_tensor("m", [C, C], f16, kind="ExternalInput")
    wvT = nc.dram_tensor("wvT", [C, C], f16, kind="ExternalInput")
    wud = nc.dram_tensor("wu", [C, 1], f32, kind="ExternalInput")
    out = nc.dram_tensor("out", [C, N], f16, kind="ExternalOutput")

    with tile.TileContext(nc) as tc, ExitStack() as ctx:
        consts = ctx.enter_context(tc.tile_pool(name="consts", bufs=1))
        big = ctx.enter_context(tc.tile_pool(name="big", bufs=1))

        m_sb = consts.tile([128, CK, C], f16, name="m_sb")
        wv_sb = consts.tile([128, CK, C], f16, name="wv_sb")
        wu_sb = consts.tile([128, CK, 1], f32, name="wu_sb")
        ones_sb = consts.tile([128, CK, 128], f8, name="ones_sb")
        warm_in = consts.tile([128, 1], f32, name="warm_in")
        warm_out = consts.tile([128, 1], f16, name="warm_out")

        u_cs = [
            big.tile([128, CK, 512], f8, name=f"u{t}", tag=f"u{t}")
            for t in range(NT)
        ]
        rgb16_cs = [
            big.tile([128, CK, 256], f16, name=f"rgb16_{g}", tag=f"rgb16_{g}")
            for g in range(NJP)
        ]
        rgb8_cs = [
            big.tile([128, CK, 256], f8, name=f"rgb8_{g}", tag=f"rgb8_{g}")
            for g in range(NJP)
        ]
        # v^T resident: v_sb[p, jb, c] = 32*v[c, jb*128+p]
        v_sb = big.tile([128, NJ, C], f8, name="v_sb")

        pt_pool = ctx.enter_context(tc.tile_pool(name="pt", bufs=20))
        spool = ctx.enter_context(tc.tile_pool(name="spsum", bufs=2, space="PSUM"))
        small = ctx.enter_context(tc.tile_pool(name="small", bufs=4))

        def emit_s(it, jp):
            """S^T psum for j-blocks (2jp, 2jp+1) x i-tile it, one paired exp.

            Each half is one DoubleRow matmul: contraction over all 256
            channels at once (lhsT = rgb8 pair-chunk, rhs = u8 pair-tile).
            """
            ps = spool.tile([128, 2, 512], f32, name="ps_s", tag="s")
            smm = None
            for b in range(2):
                jb = 2 * jp + b
                smm = nc.tensor.matmul(
                    ps[:, b, :],
                    lhsT=rgb8_cs[jb // 2][:, :, (jb % 2) * 128 : (jb % 2 + 1) * 128],
                    rhs=u_cs[it],
                    start=True,
                    stop=True,
                    perf_mode=DR,
                )
            pt = pt_pool.tile([128, 2, 512], f8, name="pt", tag="pt")
            nc.scalar.activation(
                pt, ps, mybir.ActivationFunctionType.Exp, scale=1.0 / SC
            )
            return pt, smm

        with (
            tc.tile_pool(name="io", bufs=4) as io,
            tc.tile_pool(name="ppsum", bufs=4, space="PSUM") as pp,
        ):
            # Critical-path DMAs on the sync queue, in consumption order:
            # Q(0) needs m+hsv0(+wu); the first V matmul needs wv+rgb16[0].
            nc.sync.dma_start(out=m_sb[:], in_=md.rearrange("(k p) m -> p k m", p=128))
            xhs = [io.tile([128, CK, 512], f16, name="xh", tag="xh", bufs=8)
                   for _ in range(NT)]
            nc.sync.dma_start(out=xhs[0][:], in_=hsv[0])
            nc.sync.dma_start(out=wu_sb[:], in_=wud.rearrange("(k p) o -> p k o", p=128))
            nc.sync.dma_start(out=wv_sb[:], in_=wvT.rearrange("(k p) m -> p k m", p=128))
            nc.sync.dma_start(out=rgb16_cs[0][:], in_=rgb16[0])
            # prefetch heads on the gpsimd queue; the rest is paced in-loop
            for g8 in range(3):
                nc.gpsimd.dma_start(out=rgb8_cs[g8][:], in_=rgb8[g8])
            for g16 in range(1, 4):
                nc.gpsimd.dma_start(out=rgb16_cs[g16][:], in_=rgb16[g16])
            for t in (1, 2):
                nc.scalar.dma_start(out=xhs[t][:], in_=hsv[t])
            # pre-load the exp spline table under the DMAs
            nc.vector.memset(warm_in[:], 0.0)
            nc.scalar.activation(warm_out, warm_in, mybir.ActivationFunctionType.Exp)
            nc.vector.memset(ones_sb[:], SC)

            # Q pass: u' = m^T hsv + wu (fp16 matmul, fp8 store)
            def emit_q(t):
                for ci in range(CK):
                    ps = pp.tile([128, 512], f32, name="ps_q", tag="pp")
                    for k in range(CK):
                        nc.tensor.matmul(
                            ps,
                            lhsT=m_sb[:, k, ci * 128 : (ci + 1) * 128],
                            rhs=xhs[t][:, k, :],
                            start=(k == 0),
                            stop=(k == CK - 1),
                        )
                    nc.vector.tensor_scalar_add(
                        u_cs[t][:, ci, :], ps, wu_sb[:, ci, :]
                    )
                if t + 2 < NT:
                    nc.scalar.dma_start(out=xhs[t + 2][:], in_=hsv[t + 2])

            emit_q(0)
            # Interleave S(0)/exp (starts the ScalarE stream ASAP) with the
            # V projection and the remaining Q projections.
            cur = []
            t_next = 1
            for jp in range(NJP):
                pt, smm = emit_s(0, jp)
                cur.append(pt)
                for b in range(2):
                    j = 2 * jp + b
                    g, half = j // 2, j % 2
                    ps = pp.tile([128, C], f32, name="ps_v", tag="pp")
                    for k in range(CK):
                        nc.tensor.matmul(
                            ps,
                            lhsT=rgb16_cs[g][:, k, half * 128 : (half + 1) * 128],
                            rhs=wv_sb[:, k, :],
                            start=(k == 0),
                            stop=(k == CK - 1),
                        )
                    nc.vector.tensor_copy(v_sb[:, j, :], ps)
                if jp % 2 == 1 and t_next < NT:
                    emit_q(t_next)
                    t_next += 1
                # paced prefetch of the remaining rgb chunks
                if jp + 3 < NJP:
                    dma = nc.gpsimd.dma_start(
                        out=rgb8_cs[jp + 3][:], in_=rgb8[jp + 3]
                    )
                    _add_dep_helper(
                        dma.ins, smm.ins, sync=True,
                        reason="throttle rgb8 prefetch behind S matmuls",
                    )
                if jp + 4 < NJP:
                    dma = nc.gpsimd.dma_start(
                        out=rgb16_cs[jp + 4][:], in_=rgb16[jp + 4]
                    )
                    _add_dep_helper(
                        dma.ins, smm.ins, sync=True,
                        reason="throttle rgb16 prefetch behind S matmuls",
                    )

        opool = ctx.enter_context(tc.tile_pool(name="opsum", bufs=1, space="PSUM"))

        for it in range(NT):
            pv = [
                opool.tile([128, 512], f32, name=f"pv{cb}", tag=f"pv{cb}")
                for cb in range(CK)
            ]
            den = opool.tile([128, 512], f32, name="den", tag="den", bufs=2)
            nxt = [None] * NJP
            for jp in range(NJP):
                # S of the next i-tile first: covers the psum-reuse stall of
                # pv/den at the i-tile boundary (PE queue is FIFO)
                if it + 1 < NT:
                    nxt[jp], _ = emit_s(it + 1, jp)
                nc.tensor.matmul(
                    den,
                    lhsT=ones_sb[:],
                    rhs=cur[jp],
                    start=(jp == 0),
                    stop=(jp == NJP - 1),
                    perf_mode=DR,
                )
                for cb in range(CK):
                    nc.tensor.matmul(
                        pv[cb],
                        lhsT=v_sb[:, 2 * jp : 2 * jp + 2, cb * 128 : (cb + 1) * 128],
                        rhs=cur[jp],
                        start=(jp == 0),
                        stop=(jp == NJP - 1),
                        perf_mode=DR,
                    )
            # tail: spill pv psums to SBUF fast (frees banks for tile it+1),
            # then normalize and emit
            pvs = []
            for cb in range(CK):
                pvsb = small.tile([128, 512], f32, name=f"pvs{cb}",
                                  tag=f"pvs{cb}", bufs=2)
                nc.vector.tensor_copy(pvsb, pv[cb])
                pvs.append(pvsb)
            rec = small.tile([128, 512], f32, name="rec", tag="rec", bufs=2)
            nc.vector.reciprocal_approx_fast(rec, den)
            for cb in range(CK):
                ot = small.tile([128, 512], f16, name="ot", tag="ot", bufs=4)
                nc.vector.tensor_mul(ot, pvs[cb], rec)
                eng = nc.sync if cb == 0 else nc.gpsimd
                eng.dma_start(
                    out=out[cb * 128 : (cb + 1) * 128, it * 512 : (it + 1) * 512],
                    in_=ot,
                )
            cur = nxt

    nc.compile()
    return nc


def _get_nc():
    if "nc" not in _CACHE:
        _CACHE["nc"] = _build()
    return _CACHE["nc"]


def kernel(rgb_feat, hsv_feat, Wq, bq, Wk, bk, Wv, bv, _debug=None):
    import ml_dtypes
    from concourse.bass_utils import run_bass_kernel_spmd

    F8 = ml_dtypes.float8_e4m3  # TRN fp8e4: e4m3 with inf, max 240

    rgb32 = np.asarray(rgb_feat, np.float32)
    hsv32 = np.asarray(hsv_feat, np.float32)
    # pack: [b, C, h, w] -> chunked partition-contiguous layouts
    hsv_p = np.ascontiguousarray(
        hsv32.reshape(B, CK, 128, NT, 512).transpose(0, 3, 2, 1, 4)
    ).astype(np.float16)  # [b, NT, 128, CK, 512]
    rgb_p = np.ascontiguousarray(
        rgb32.reshape(B, CK, 128, NJP, 256).transpose(0, 3, 2, 1, 4)
    )  # [b, NJP, 128, CK, 256]
    rgb16_p = rgb_p.astype(np.float16)
    rgb8_p = np.clip(rgb_p, -240, 240).astype(F8)

    scale = (np.float32(1.0) / np.sqrt(np.float32(C))) * np.float32(SC)
    Wq32 = np.asarray(Wq, np.float32)
    Wk32 = np.asarray(Wk, np.float32)
    m_ = np.ascontiguousarray(((Wq32.T @ Wk32) * scale).astype(np.float16))
    wu_ = np.ascontiguousarray(
        ((Wk32.T @ np.asarray(bq, np.float32)) * scale).reshape(C, 1)
    ).astype(np.float32)
    wvT16 = np.ascontiguousarray(
        (np.asarray(Wv, np.float32).T * np.float32(SC)).astype(np.float16)
    )
    bv_col = np.asarray(bv, np.float32).reshape(C, 1)

    in_maps = []
    for bi in range(B):
        in_maps.append(
            {
                "hsv": hsv_p[bi],
                "rgb16": rgb16_p[bi],
                "rgb8": rgb8_p[bi],
                "m": m_,
                "wvT": wvT16,
                "wu": wu_,
            }
        )

    nc = _get_nc()
    kwargs = dict(_debug or {})
    kwargs.pop("result", None)
    res = run_bass_kernel_spmd(nc, in_maps, core_ids=list(range(B)), **kwargs)
    if _debug is not None:
        _debug["result"] = res
    outs = [
        (res.results[bi]["out"].astype(np.float32) + bv_col).reshape(C, H, W)
        for bi in range(B)
    ]
    return np.stack(outs, axis=0).astype(np.float32)


# revision 10
# speedup vs baseline: 1.4543x; 1.0311x over previous
"""Cross-attention Bass kernel for Trainium2, data-parallel over batch.

Problem (hardcoded): b=8, c=256, h=w=64 (n=4096).
  q = Wq@hsv + bq; k = Wk@rgb + bk; v = Wv@rgb + bv   (1x1 convs, [c, n])
  attn = softmax_j(q_i . k_j / sqrt(c)); out[c,i] = sum_j v[c,j] attn[i,j]

Per-core design (one batch per NeuronCore, 8 cores):
  - Weight folding (host): m = (Wq^T Wk)/sqrt(c) so S^T = rgb^T (m^T hsv);
    wu = (Wk^T bq)/sqrt(c) folds the bq cross-term into u; the q.bk and
    bq.bk terms are row-uniform in the softmax and cancel exactly.
  - fp8e4m3 DoubleRow matmuls for the two big GEMMs (S and PV) and the
    softmax denominator: contraction 256 in one PE pass at 2 fp8
    MACs/cell/cycle. A 2^5 scale rides on m/wu (removed by the exp's free
    scale) so fp8 operands stay out of the subnormal range; Wv carries the
    same 2^5 which cancels against the 2^5-valued ones-weights of the
    denominator matmul.
  - Projections stay fp16 (~14us PE total) so quantization error is
    confined to u/P/V; measured end-to-end rel err ~1.6e-2 (gate 2e-2).
  - S^T layout: psum S^T[j, 2jb, i] so exp feeds PV directly: the exp
    output pt [128, 2, 512] fp8 is exactly a DoubleRow rhs.
  - PV runs V^T-stationary: lhsT = v^T pair-blocks [128, 2, 128], rhs = pt;
    out lands as out[c, i] (natural [C, N] layout, no transposes anywhere).
    The denominator is a third pass over pt with constant 32.0 weights,
    giving a partition-broadcast denominator for the final division.
  - ScalarE is the critical engine (128 exps of FD=1024, ~147us); the
    prologue is arranged so the exp stream starts within ~4us: exp table
    pre-loaded via a dummy ACT under the first DMAs, Q(0) projected first,
    and S(0)/exp interleaved with the V projection.
  - Tail per i-tile: pv psums spill to SBUF on DVE (frees PSUM banks fast),
    denominator reciprocal via reciprocal_approx_fast (~18 bits, plenty
    for a well-conditioned denominator), divide+fp16-store on DVE, DMA out.
"""

import numpy as np

B, C, H, W = 8, 256, 64, 64
N = H * W          # 4096
CK = C // 128      # 2 channel chunks
NJ = N // 128      # 32 key blocks
NJP = NJ // 2      # 16 paired key blocks
NT = N // 512      # 8 query tiles of 512
SC = 32.0          # 2^5 fp8 anti-subnormal boost

_CACHE = {}


def _build():
    import concourse.tile as tile
    from concourse import bacc, mybir
    from contextlib import ExitStack
    from concourse.bass import _add_dep_helper

    f32 = mybir.dt.float32
    f16 = mybir.dt.float16
    f8 = mybir.dt.float8e4
    DR = mybir.MatmulPerfMode.DoubleRow

    nc = bacc.Bacc(None, target_bir_lowering=False)

    # host-packed inputs; every chunk DMA is contiguous per partition
    hsv = nc.dram_tensor("hsv", [NT, 128, CK, 512], f16, kind="ExternalInput")
    rgb16 = nc.dram_tensor("rgb16", [NJP, 128, CK, 256], f16, kind="ExternalInput")
    rgb8 = nc.dram_tensor("rgb8", [NJP, 128, CK, 256], f8, kind="ExternalInput")
    md = nc.dram_tensor("m", [C, C], f16, kind="ExternalInput")
    wvT = nc.dram_tensor("wvT", [C, C], f16, kind="ExternalInput")
    wud = nc.dram_tensor("wu", [C, 1], f32, kind="ExternalInput")
    # out[cb, it, p, i] = result[cb*128+p, it*512+i]: each [128, 512] store
    # is one fully-contiguous 128KB burst (the host re-assembles)
    out = nc.dram_tensor("out", [CK, NT, 128, 512], f16, kind="ExternalOutput")

    with tile.TileContext(nc) as tc, ExitStack() as ctx:
        consts = ctx.enter_context(tc.tile_pool(name="consts", bufs=1))
        big = ctx.enter_context(tc.tile_pool(name="big", bufs=1))

        m_sb = consts.tile([128, CK, C], f16, name="m_sb")
        wv_sb = consts.tile([128, CK, C], f16, name="wv_sb")
        wu_sb = consts.tile([128, CK, 1], f32, name="wu_sb")
        ones_sb = consts.tile([128, CK, 128], f8, name="ones_sb")
        warm_in = consts.tile([128, 1], f32, name="warm_in")
        warm_out = consts.tile([128, 1], f16, name="warm_out")

        u_cs = [
            big.tile([128, CK, 512], f8, name=f"u{t}", tag=f"u{t}")
            for t in range(NT)
        ]
        rgb16_cs = [
            big.tile([128, CK, 256], f16, name=f"rgb16_{g}", tag=f"rgb16_{g}")
            for g in range(NJP)
        ]
        rgb8_cs = [
            big.tile([128, CK, 256], f8, name=f"rgb8_{g}", tag=f"rgb8_{g}")
            for g in range(NJP)
        ]
        # v^T resident: v_sb[p, jb, c] = 32*v[c, jb*128+p]
        v_sb = big.tile([128, NJ, C], f8, name="v_sb")

        pt_pool = ctx.enter_context(tc.tile_pool(name="pt", bufs=20))
        spool = ctx.enter_context(tc.tile_pool(name="spsum", bufs=2, space="PSUM"))
        small = ctx.enter_context(tc.tile_pool(name="small", bufs=4))

        def emit_s(it, jp):
            """S^T psum for j-blocks (2jp, 2jp+1) x i-tile it, one paired exp.

            Each half is one DoubleRow matmul: contraction over all 256
            channels at once (lhsT = rgb8 pair-chunk, rhs = u8 pair-tile).
            """
            ps = spool.tile([128, 2, 512], f32, name="ps_s", tag="s")
            smm = None
            for b in range(2):
                jb = 2 * jp + b
                smm = nc.tensor.matmul(
                    ps[:, b, :],
                    lhsT=rgb8_cs[jb // 2][:, :, (jb % 2) * 128 : (jb % 2 + 1) * 128],
                    rhs=u_cs[it],
                    start=True,
                    stop=True,
                    perf_mode=DR,
                )
            pt = pt_pool.tile([128, 2, 512], f8, name="pt", tag="pt")
            nc.scalar.activation(
                pt, ps, mybir.ActivationFunctionType.Exp, scale=1.0 / SC
            )
            return pt, smm

        with (
            tc.tile_pool(name="io", bufs=4) as io,
            tc.tile_pool(name="ppsum", bufs=4, space="PSUM") as pp,
        ):
            # Critical-path DMAs on the sync queue, in consumption order:
            # Q(0) needs m+hsv0(+wu); the first V matmul needs wv+rgb16[0].
            nc.sync.dma_start(out=m_sb[:], in_=md.rearrange("(k p) m -> p k m", p=128))
            xhs = [io.tile([128, CK, 512], f16, name="xh", tag="xh", bufs=8)
                   for _ in range(NT)]
            hsv0_dma = nc.sync.dma_start(out=xhs[0][:], in_=hsv[0])
            nc.sync.dma_start(out=wu_sb[:], in_=wud.rearrange("(k p) o -> p k o", p=128))
            nc.sync.dma_start(out=wv_sb[:], in_=wvT.rearrange("(k p) m -> p k m", p=128))
            nc.sync.dma_start(out=rgb16_cs[0][:], in_=rgb16[0])
            # prefetch heads on the gpsimd queue; the rest is paced in-loop.
            # Only the first two rgb8 chunks race the critical sync DMAs;
            # the rest wait for hsv[0] so Q(0) (-> first exp) isn't delayed.
            for g8 in range(2):
                nc.gpsimd.dma_start(out=rgb8_cs[g8][:], in_=rgb8[g8])
            for g8 in range(2, 5):
                dma = nc.gpsimd.dma_start(out=rgb8_cs[g8][:], in_=rgb8[g8])
                _add_dep_helper(
                    dma.ins, hsv0_dma.ins, sync=True,
                    reason="keep HBM clear for the critical hsv0 load",
                )
            for g16 in range(1, 6):
                dma = nc.gpsimd.dma_start(out=rgb16_cs[g16][:], in_=rgb16[g16])
                _add_dep_helper(
                    dma.ins, hsv0_dma.ins, sync=True,
                    reason="keep HBM clear for the critical hsv0 load",
                )
            for t in (1, 2):
                nc.scalar.dma_start(out=xhs[t][:], in_=hsv[t])
            # pre-load the exp spline table under the DMAs
            nc.vector.memset(warm_in[:], 0.0)
            nc.scalar.activation(warm_out, warm_in, mybir.ActivationFunctionType.Exp)
            nc.vector.memset(ones_sb[:], SC)

            # Q pass: u' = m^T hsv + wu (fp16 matmul, fp8 store)
            def emit_q(t):
                for ci in range(CK):
                    ps = pp.tile([128, 512], f32, name="ps_q", tag="pp")
                    for k in range(CK):
                        nc.tensor.matmul(
                            ps,
                            lhsT=m_sb[:, k, ci * 128 : (ci + 1) * 128],
                            rhs=xhs[t][:, k, :],
                            start=(k == 0),
                            stop=(k == CK - 1),
                        )
                    nc.vector.tensor_scalar_add(
                        u_cs[t][:, ci, :], ps, wu_sb[:, ci, :]
                    )
                if t + 2 < NT:
                    nc.scalar.dma_start(out=xhs[t + 2][:], in_=hsv[t + 2])

            emit_q(0)
            # Interleave S(0)/exp (starts the ScalarE stream ASAP) with the
            # V projection and the remaining Q projections.
            cur = []
            t_next = 1
            for jp in range(NJP):
                pt, smm = emit_s(0, jp)
                cur.append(pt)
                for b in range(2):
                    j = 2 * jp + b
                    g, half = j // 2, j % 2
                    ps = pp.tile([128, C], f32, name="ps_v", tag="pp")
                    for k in range(CK):
                        nc.tensor.matmul(
                            ps,
                            lhsT=rgb16_cs[g][:, k, half * 128 : (half + 1) * 128],
                            rhs=wv_sb[:, k, :],
                            start=(k == 0),
                            stop=(k == CK - 1),
                        )
                    nc.vector.tensor_copy(v_sb[:, j, :], ps)
                if jp % 2 == 1 and t_next < NT:
                    emit_q(t_next)
                    t_next += 1
                # paced prefetch of the remaining rgb chunks
                if jp + 5 < NJP:
                    dma = nc.gpsimd.dma_start(
                        out=rgb8_cs[jp + 5][:], in_=rgb8[jp + 5]
                    )
                    _add_dep_helper(
                        dma.ins, smm.ins, sync=True,
                        reason="throttle rgb8 prefetch behind S matmuls",
                    )
                if jp + 6 < NJP:
                    dma = nc.gpsimd.dma_start(
                        out=rgb16_cs[jp + 6][:], in_=rgb16[jp + 6]
                    )
                    _add_dep_helper(
                        dma.ins, smm.ins, sync=True,
                        reason="throttle rgb16 prefetch behind S matmuls",
                    )

        opool = ctx.enter_context(tc.tile_pool(name="opsum", bufs=1, space="PSUM"))

        for it in range(NT):
            pv = [
                opool.tile([128, 512], f32, name=f"pv{cb}", tag=f"pv{cb}")
                for cb in range(CK)
            ]
            den = opool.tile([128, 512], f32, name="den", tag="den", bufs=2)
            nxt = [None] * NJP
            for jp in range(NJP):
                # S of the next i-tile first: covers the psum-reuse stall of
                # pv/den at the i-tile boundary (PE queue is FIFO)
                if it + 1 < NT:
                    nxt[jp], _ = emit_s(it + 1, jp)
                nc.tensor.matmul(
                    den,
                    lhsT=ones_sb[:],
                    rhs=cur[jp],
                    start=(jp == 0),
                    stop=(jp == NJP - 1),
                    perf_mode=DR,
                )
                for cb in range(CK):
                    nc.tensor.matmul(
                        pv[cb],
                        lhsT=v_sb[:, 2 * jp : 2 * jp + 2, cb * 128 : (cb + 1) * 128],
                        rhs=cur[jp],
                        start=(jp == 0),
                        stop=(jp == NJP - 1),
                        perf_mode=DR,
                    )
            # tail: spill pv psums to SBUF fast (frees banks for tile it+1),
            # then normalize and emit
            pvs = []
            for cb in range(CK):
                pvsb = small.tile([128, 512], f32, name=f"pvs{cb}",
                                  tag=f"pvs{cb}", bufs=2)
                nc.vector.tensor_copy(pvsb, pv[cb])
                pvs.append(pvsb)
            rec = small.tile([128, 512], f32, name="rec", tag="rec", bufs=2)
            nc.vector.reciprocal_approx_fast(rec, den)
            for cb in range(CK):
                ot = small.tile([128, 512], f16, name="ot", tag="ot", bufs=4)
                nc.vector.tensor_mul(ot, pvs[cb], rec)
                eng = nc.sync if cb == 0 else nc.gpsimd
                eng.dma_start(out=out[cb, it], in_=ot)
            cur = nxt

    nc.compile()
    return nc


def _get_nc():
    if "nc" not in _CACHE:
        _CACHE["nc"] = _build()
    return _CACHE["nc"]


def kernel(rgb_feat, hsv_feat, Wq, bq, Wk, bk, Wv, bv, _debug=None):
    import ml_dtypes
    from concourse.bass_utils import run_bass_kernel_spmd

    F8 = ml_dtypes.float8_e4m3  # TRN fp8e4: e4m3 with inf, max 240

    rgb32 = np.asarray(rgb_feat, np.float32)
    hsv32 = np.asarray(hsv_feat, np.float32)
    # pack: [b, C, h, w] -> chunked partition-contiguous layouts
    hsv_p = np.ascontiguousarray(
        hsv32.reshape(B, CK, 128, NT, 512).transpose(0, 3, 2, 1, 4)
    ).astype(np.float16)  # [b, NT, 128, CK, 512]
    rgb_p = np.ascontiguousarray(
        rgb32.reshape(B, CK, 128, NJP, 256).transpose(0, 3, 2, 1, 4)
    )  # [b, NJP, 128, CK, 256]
    rgb16_p = rgb_p.astype(np.float16)
    rgb8_p = np.clip(rgb_p, -240, 240).astype(F8)

    scale = (np.float32(1.0) / np.sqrt(np.float32(C))) * np.float32(SC)
    Wq32 = np.asarray(Wq, np.float32)
    Wk32 = np.asarray(Wk, np.float32)
    m_ = np.ascontiguousarray(((Wq32.T @ Wk32) * scale).astype(np.float16))
    wu_ = np.ascontiguousarray(
        ((Wk32.T @ np.asarray(bq, np.float32)) * scale).reshape(C, 1)
    ).astype(np.float32)
    wvT16 = np.ascontiguousarray(
        (np.asarray(Wv, np.float32).T * np.float32(SC)).astype(np.float16)
    )
    bv_col = np.asarray(bv, np.float32).reshape(C, 1)

    in_maps = []
    for bi in range(B):
        in_maps.append(
            {
                "hsv": hsv_p[bi],
                "rgb16": rgb16_p[bi],
                "rgb8": rgb8_p[bi],
                "m": m_,
                "wvT": wvT16,
                "wu": wu_,
            }
        )

    nc = _get_nc()
    kwargs = dict(_debug or {})
    kwargs.pop("result", None)
    res = run_bass_kernel_spmd(nc, in_maps, core_ids=list(range(B)), **kwargs)
    if _debug is not None:
        _debug["result"] = res
    outs = []
    for bi in range(B):
        o = res.results[bi]["out"]  # [CK, NT, 128, 512] f16
        o = o.transpose(0, 2, 1, 3).reshape(C, N).astype(np.float32) + bv_col
        outs.append(o.reshape(C, H, W))
    return np.stack(outs, axis=0).astype(np.float32)
